# revision 33
# baseline (speedup 1.0000x reference)
"""Trainium2 Bass kernel for DenseLayerWithComplexNeurons.

Reference computation (B=8, S=1024, DIN=1024, DOUT=1024, A=4, T=4, H=8):
    z = x @ W.T + bias                      # (B,S, A*DOUT)
    z -> (B,S,T,G,A), G = DOUT//T = 256
    h = tanh(z @ cw1[t] + cb1[t])           # (B,S,T,G,H)
    o = h @ cw2[t] + cb2[t]                 # (B,S,T,G) -> (B,S,DOUT)

Sharding: 8 cores = 4 token blocks (2048 tokens each) x 2 feature halves
(2048 W-rows / 512 neurons each).  All compute runs in a transposed layout
(features on partitions, tokens on the free dim) so the tiny per-neuron
MLPs become small constant matmuls on the tensor engine:
  - expansion E[t]: (g,a) -> (g,h) block-diagonal with cw1
  - reduction S[t]: (g,h) -> (g)   block-diagonal with cw2
The linear bias and cb1 are folded through cw1 into a single per-feature
bias bb added by the scalar engine inside tanh.  All matmul operands are
fp16: same PE column rate as fp32r, but LDWEIGHTS uses fast-weight-load
(2 elems per 32b read) so the weight loads hide behind the matmuls, and
DMA traffic halves.
"""

import numpy as np

import concourse.bass as bass  # noqa: F401  (bass types via bacc)
import concourse.mybir as mybir
import concourse.tile as tile
from concourse import bacc
from concourse.bass_utils import run_bass_kernel_spmd

F32 = mybir.dt.float32
F16 = mybir.dt.float16

B, S, DIN, DOUT, A, T, H = 8, 1024, 1024, 1024, 4, 4, 8
G = DOUT // T                     # 256 neurons per cell type
NTOK = B * S                      # 8192 tokens
DP, TP = 4, 2                     # token blocks x feature halves
TOK_C = NTOK // DP                # 2048 tokens per core
NRN_C = DOUT // TP                # 512 neurons per core
FEAT_C = A * NRN_C                # 2048 A-expanded features per core
KC = DIN // 128                   # 8 contraction chunks
NB = TOK_C // 512                 # 4 token sub-blocks per core
FC = FEAT_C // 128                # 16 feature chunks per core
TL = FC // 2                      # 8 feature chunks per cell type


_NC_CACHE = []


def _build_nc():
    nc = bacc.Bacc("TRN2", target_bir_lowering=False, debug=False, num_devices=8)

    # layouts chosen so every DMA piece is one fully-contiguous DRAM
    # region: xT[nb, i, p, j] (piece i = 512 tokens) and wT[fc, p, f]
    xT = nc.declare_dram_parameter("xT", [NB, 8, 128, 512], F16,
                                   isOutput=False)
    wT = nc.declare_dram_parameter("wT", [FC, 128, KC * 128], F16,
                                   isOutput=False)
    eM = nc.declare_dram_parameter("eM", [128, 2, 128], F16, isOutput=False)
    sM = nc.declare_dram_parameter("sM", [128, 2, 8, 128], F16, isOutput=False)
    bb = nc.declare_dram_parameter("bb", [128, 2 * FC], F32, isOutput=False)
    c2 = nc.declare_dram_parameter("c2", [128, 2], F32, isOutput=False)
    oT = nc.declare_dram_parameter("oT", [NRN_C, TOK_C], F32, isOutput=True)

    with tile.TileContext(nc) as tc:
        with tc.tile_pool(name="wp", bufs=1) as wp, \
             tc.tile_pool(name="cst", bufs=1) as cst, \
             tc.tile_pool(name="xp", bufs=4) as xp, \
             tc.tile_pool(name="zb", bufs=6) as zb, \
             tc.tile_pool(name="tb", bufs=18) as tb, \
             tc.tile_pool(name="ob", bufs=2) as ob, \
             tc.tile_pool(name="zp", bufs=2, space="PSUM") as zp, \
             tc.tile_pool(name="hp", bufs=4, space="PSUM") as hp, \
             tc.tile_pool(name="op", bufs=2, space="PSUM") as op:

            # --- PE warm-up + ACT table preload during the input DMA window
            wu_f = cst.tile([128, 512], F32, tag="wuf")
            nc.vector.memset(wu_f[:], 0.001)
            wu = cst.tile([128, 512], F16, tag="wu")
            nc.vector.tensor_copy(wu[:], wu_f[:])
            nc.scalar.activation(wu_f[:, 0:8], wu_f[:, 8:16],
                                 mybir.ActivationFunctionType.Tanh)
            for _ in range(14):
                wu_ps = zp.tile([128, 512], F32, tag="z")
                nc.tensor.matmul(wu_ps[:], wu[:, 0:128], wu[:],
                                 start=True, stop=True)

            # --- inputs on the sync ring.  The first z group needs x0 +
            # w[fc0]; issue those first, split so the pieces spread across
            # the HW queues, then stream the rest of w.  Small consts go
            # on the scalar ring, which drains before tanh work starts.
            x0 = xp.tile([128, KC * 512], F16, tag="x")
            x_tiles = [x0]
            for nb in range(1, NB):
                x_tiles.append(xp.tile([128, KC * 512], F16, tag="x",
                                       name=f"x_{nb}"))
            w_all = wp.tile([128, FC, KC * 128], F16, tag="w")
            s_sb = cst.tile([128, 2, 8, 128], F16, tag="s")
            for i in range(4):     # w[fc0] in 4 contiguous partition-bands
                pp = bass.ds(i * 32, 32)
                nc.sync.dma_start(w_all[pp, 0], wT[0, pp])
            for i in range(8):     # x0 in 8 contiguous pieces
                nc.sync.dma_start(x0[:, bass.ds(i * 512, 512)], xT[0, i])
            for i in range(4):     # w[fc1]
                pp = bass.ds(i * 32, 32)
                nc.sync.dma_start(w_all[pp, 1], wT[1, pp])
            for fc in (2, 3):      # earliest-needed chunks in halves
                for i in range(2):
                    pp = bass.ds(i * 64, 64)
                    nc.sync.dma_start(w_all[pp, fc], wT[fc, pp])
            for fc in range(4, FC):
                nc.sync.dma_start(w_all[:, fc], wT[fc])
                if fc == 5:   # s halves slot in behind the w chunks that the
                    nc.sync.dma_start(s_sb[:, 0], sM[:, 0])
                if fc == 9:   # first (second) cell type's S-matmuls need
                    nc.sync.dma_start(s_sb[:, 1], sM[:, 1])
            for nb in (1, 2, 3):   # x prefetches after the full w stream
                for i in range(KC):
                    nc.sync.dma_start(
                        x_tiles[nb][:, bass.ds(i * 512, 512)], xT[nb, i])
            e_sb = cst.tile([128, 2, 128], F16, tag="e")
            nc.scalar.dma_start(e_sb[:], eM[:])
            bb_sb = cst.tile([128, 2 * FC], F32, tag="bb")
            nc.scalar.dma_start(bb_sb[:], bb[:])
            c2_sb = cst.tile([128, 2], F32, tag="c2")
            nc.scalar.dma_start(c2_sb[:], c2[:])

            # Software-pipelined epilogue: for each 128-feature chunk the
            # expansion (E) matmuls run one chunk behind the mains; the
            # reduction for a full (nb, grp) unit runs once all 8 of its
            # tanh tiles exist, so the PE never waits on the DVE cast or
            # the ScalarE tanh.
            unit_ths = {}                      # (nb, grp) -> [th] * 8
            e_stage = []                       # awaiting expansion
            s_stage = []                       # (unit, qslot) awaiting S
            o_next = [None]                    # pre-cleared o_ps for next unit
            qnow = [0]                         # global q-slot counter

            def alloc_clear_ops():
                # Pre-clear the next unit's PSUM bank on the DVE (one unit
                # ahead, so the PE never waits): with data == 0, the
                # reduction matmuls can all run start=False -- accumulate
                # and overwrite are then equivalent per element.
                t = op.tile([128, 512], F32, tag="o")
                nc.vector.memset(t[:], 0.0)
                return t

            def emit_expansion(item):
                nb, grp, q, tl, z_sb = item
                fc = grp * 4 + q
                ths = unit_ths.setdefault((nb, grp), [])
                for half in range(2):
                    ci = fc * 2 + half
                    # K=64 row-tiled pair: base_partition 0/64 auto-derives
                    # tile_position -> both halves run concurrently.
                    h_ps = hp.tile([128, 512], F32, tag="h")
                    nc.tensor.matmul(
                        h_ps[:],
                        e_sb[bass.ds(half * 64, 64), tl, :],
                        z_sb[bass.ds(half * 64, 64), :],
                        start=True, stop=True)
                    th = tb.tile([128, 512], F16, tag="t")
                    nc.scalar.activation(
                        th[:], h_ps[:],
                        mybir.ActivationFunctionType.Tanh,
                        bias=bb_sb[:, bass.ds(ci, 1)])
                    ths.append(th)
                if q == 3:
                    s_stage.append(
                        ((nb, grp, tl, unit_ths.pop((nb, grp))), qnow[0]))

            def emit_reduction(item):
                # Full reduction for one (nb, grp) unit: 8 th tiles, each
                # holding 16 neurons x 8 h on its partitions.  All 8 rr run
                # as M=32 column-tiled matmuls into the pre-zeroed bank:
                # each only touches its own 32-column group of the PE array,
                # so the 4 column groups execute concurrently (2 waves
                # instead of 8 serial slots).
                nb, grp, tl, ths = item
                if o_next[0] is None:
                    o_next[0] = alloc_clear_ops()
                o_ps = o_next[0]
                for rr in (0, 2, 4, 6, 1, 3, 5, 7):  # 2 waves of 4 col grps
                    j = rr // 2
                    nc.tensor.matmul(
                        o_ps[bass.ds(32 * j, 32), :],
                        s_sb[:, tl, rr, bass.ds(32 * j, 32)],
                        ths[rr][:],
                        start=False, stop=(rr == 7), skip_group_check=True,
                        tile_position=(0, 32 * j))
                o_next[0] = alloc_clear_ops()
                o_sb = ob.tile([128, 512], F32, tag="o")
                last = (nb == NB - 1 and grp == 3)
                if last:
                    # tail chain: use the idle ScalarE for the cb2 add and
                    # the lower-latency HWDGE sync ring, split over 4 queues
                    nc.scalar.activation(
                        o_sb[:], o_ps[:],
                        mybir.ActivationFunctionType.Identity,
                        bias=c2_sb[:, bass.ds(tl, 1)])
                    for i in range(4):
                        nc.sync.dma_start(
                            oT[bass.ds(grp * 128, 128),
                               bass.ds(nb * 512 + i * 128, 128)],
                            o_sb[:, bass.ds(i * 128, 128)])
                else:
                    nc.vector.tensor_scalar_add(
                        o_sb[:], o_ps[:], c2_sb[:, bass.ds(tl, 1)])
                    nc.gpsimd.dma_start(
                        oT[bass.ds(grp * 128, 128), bass.ds(nb * 512, 512)],
                        o_sb[:])

            for nb in range(NB):
                x_nb = x_tiles[nb]

                for grp in range(4):          # 4 fc chunks -> 128 neurons
                    tl = (grp * 4) // TL
                    for q in range(4):
                        fc = grp * 4 + q
                        z_ps = zp.tile([128, 512], F32, tag="z")
                        for k in range(KC):
                            nc.tensor.matmul(
                                z_ps[:],
                                w_all[:, fc, bass.ds(k * 128, 128)],
                                x_nb[:, bass.ds(k * 512, 512)],
                                start=(k == 0), stop=(k == KC - 1))
                        z_sb = zb.tile([128, 512], F16, tag="z")
                        nc.vector.tensor_copy(z_sb[:], z_ps[:])

                        # reduction lags its unit's last tanh by >=2 q-slots
                        if s_stage and qnow[0] >= s_stage[0][1] + 2:
                            emit_reduction(s_stage.pop(0)[0])
                        # expansions go out 2 fc at a time (one mode switch
                        # amortized over 4 row-tiled K=64 matmul pairs)
                        if len(e_stage) >= 4:
                            emit_expansion(e_stage.pop(0))
                            emit_expansion(e_stage.pop(0))
                        e_stage.append((nb, grp, q, tl, z_sb))
                        qnow[0] += 1
                        # flush eagerly near the end so the final tanh
                        # batch overlaps the remaining reductions
                        if nb == NB - 1 and grp == 3 and q in (1, 3):
                            while e_stage:
                                emit_expansion(e_stage.pop(0))
                                emit_expansion(e_stage.pop(0))

            while e_stage:
                emit_expansion(e_stage.pop(0))
                emit_expansion(e_stage.pop(0))
                if s_stage:
                    emit_reduction(s_stage.pop(0)[0])
            while s_stage:
                emit_reduction(s_stage.pop(0)[0])

    nc.compile()
    return nc


def _host_prep(x, weight, bias, cw1, cb1, cw2, cb2):
    """Build the 8 per-core input maps (all host-side numpy)."""
    x2 = np.ascontiguousarray(x, dtype=np.float32).reshape(NTOK, DIN)
    weight = np.asarray(weight, dtype=np.float32)
    bias = np.asarray(bias, dtype=np.float32)
    cw1 = np.asarray(cw1, dtype=np.float32)   # (T, A, H)
    cb1 = np.asarray(cb1, dtype=np.float32)   # (T, H)
    cw2 = np.asarray(cw2, dtype=np.float32)   # (T, H)
    cb2 = np.asarray(cb2, dtype=np.float32)   # (T,)

    # xT[nb, k, p, j] = x2[tok0 + nb*512 + j, k*128 + p]
    xT_all = []
    for i in range(DP):
        blk = x2[i * TOK_C:(i + 1) * TOK_C]            # (TOK_C, DIN)
        t = blk.T.reshape(KC, 128, NB, 512)            # (k, p, nb, j)
        t = t.transpose(2, 0, 1, 3)                    # (nb, k, p, j)
        xT_all.append(np.ascontiguousarray(t, dtype=np.float16))

    # wT[fc, p, k*128 + f] = W[j*FEAT_C + fc*128 + f, k*128 + p]
    wT_all = []
    for j in range(TP):
        wj = weight[j * FEAT_C:(j + 1) * FEAT_C]       # (FEAT_C, DIN)
        t = wj.T.reshape(KC, 128, FC, 128)             # (k, p, fc, f)
        t = t.transpose(2, 1, 0, 3).reshape(FC, 128, KC * 128)
        wT_all.append(np.ascontiguousarray(t, dtype=np.float16))

    # E[t]: (g*4+a, g16*8+h) block-diag cw1; S[t]: (g*8+h, g') block-diag cw2
    e_all, s_all, bb_all, c2_all = [], [], [], []
    for j in range(TP):
        eMj = np.zeros((128, 2, 128), np.float32)
        sMj = np.zeros((128, 2, 8, 128), np.float32)
        for tl in range(2):
            t = 2 * j + tl
            for g16 in range(16):   # K=64 expansion block, doubled on rows
                for a in range(A):
                    for h in range(H):
                        v = cw1[t, a, h]
                        eMj[g16 * 4 + a, tl, g16 * 8 + h] = v
                        eMj[64 + g16 * 4 + a, tl, g16 * 8 + h] = v
            for rr in range(8):
                for g in range(16):
                    for h in range(H):
                        sMj[g * 8 + h, tl, rr, rr * 16 + g] = cw2[t, h]
        e_all.append(eMj.astype(np.float16))
        s_all.append(sMj.astype(np.float16))

        # bb[f2=(n_loc, h)] = sum_a cw1[t,a,h]*bias[t*1024+g_t*4+a] + cb1[t,h]
        nl = np.arange(NRN_C)
        t_of = (j * NRN_C + nl) // G                   # cell type per neuron
        gt = (j * NRN_C + nl) % G                      # group within type
        bias_ga = bias.reshape(T, G, A)[t_of, gt]      # (NRN_C, A)
        bbv = np.einsum('na,nah->nh', bias_ga, cw1[t_of]) + cb1[t_of]
        bb_all.append(np.ascontiguousarray(
            bbv.reshape(NRN_C * H).reshape(2 * FC, 128).T))

        c2m = np.zeros((128, 2), np.float32)
        for tl in range(2):
            c2m[:, tl] = cb2[2 * j + tl]
        c2_all.append(c2m)

    in_maps = []
    for c in range(8):
        i, j = c // TP, c % TP
        in_maps.append({
            "xT": xT_all[i], "wT": wT_all[j], "eM": e_all[j],
            "sM": s_all[j], "bb": bb_all[j], "c2": c2_all[j],
        })
    return in_maps


def kernel(x, weight, bias, cw1, cb1, cw2, cb2):
    in_maps = _host_prep(x, weight, bias, cw1, cb1, cw2, cb2)
    if not _NC_CACHE:
        _NC_CACHE.append(_build_nc())
    nc = _NC_CACHE[0]
    try:
        res = run_bass_kernel_spmd(nc, in_maps, list(range(8)))
    except Exception:
        # transient NRT device faults have been observed once after crashed
        # runs; a clean retry in the same process recovers
        res = run_bass_kernel_spmd(nc, in_maps, list(range(8)))
    out = np.empty((NTOK, DOUT), np.float32)
    for c in range(8):
        i, j = c // TP, c % TP
        oc = res.results[c]["oT"]                      # (NRN_C, TOK_C)
        out[i * TOK_C:(i + 1) * TOK_C, j * NRN_C:(j + 1) * NRN_C] = oc.T
    return out.reshape(B, S, DOUT)


# revision 34
# speedup vs baseline: 1.0185x; 1.0185x over previous
"""Trainium2 Bass kernel for DenseLayerWithComplexNeurons.

Reference computation (B=8, S=1024, DIN=1024, DOUT=1024, A=4, T=4, H=8):
    z = x @ W.T + bias                      # (B,S, A*DOUT)
    z -> (B,S,T,G,A), G = DOUT//T = 256
    h = tanh(z @ cw1[t] + cb1[t])           # (B,S,T,G,H)
    o = h @ cw2[t] + cb2[t]                 # (B,S,T,G) -> (B,S,DOUT)

Sharding: 8 cores = 4 token blocks (2048 tokens each) x 2 feature halves
(2048 W-rows / 512 neurons each).  All compute runs in a transposed layout
(features on partitions, tokens on the free dim) so the tiny per-neuron
MLPs become small constant matmuls on the tensor engine:
  - expansion E[t]: (g,a) -> (g,h) block-diagonal with cw1
  - reduction S[t]: (g,h) -> (g)   block-diagonal with cw2
The linear bias and cb1 are folded through cw1 into a single per-feature
bias bb added by the scalar engine inside tanh.  All matmul operands are
fp16: same PE column rate as fp32r, but LDWEIGHTS uses fast-weight-load
(2 elems per 32b read) so the weight loads hide behind the matmuls, and
DMA traffic halves.
"""

import numpy as np

import concourse.bass as bass  # noqa: F401  (bass types via bacc)
import concourse.mybir as mybir
import concourse.tile as tile
from concourse import bacc
from concourse.bass_utils import run_bass_kernel_spmd

F32 = mybir.dt.float32
F16 = mybir.dt.float16

B, S, DIN, DOUT, A, T, H = 8, 1024, 1024, 1024, 4, 4, 8
G = DOUT // T                     # 256 neurons per cell type
NTOK = B * S                      # 8192 tokens
DP, TP = 4, 2                     # token blocks x feature halves
TOK_C = NTOK // DP                # 2048 tokens per core
NRN_C = DOUT // TP                # 512 neurons per core
FEAT_C = A * NRN_C                # 2048 A-expanded features per core
KC = DIN // 128                   # 8 contraction chunks
NB = TOK_C // 512                 # 4 token sub-blocks per core
FC = FEAT_C // 128                # 16 feature chunks per core
TL = FC // 2                      # 8 feature chunks per cell type


_NC_CACHE = []


def _build_nc():
    nc = bacc.Bacc("TRN2", target_bir_lowering=False, debug=False, num_devices=8)

    # layouts chosen so every DMA slice is contiguous per partition:
    # xT[p, nb, k*512 + j] and wT[p, fc, k*128 + f]
    xT = nc.declare_dram_parameter("xT", [128, NB, KC * 512], F16,
                                   isOutput=False)
    wT = nc.declare_dram_parameter("wT", [128, FC, KC * 128], F16,
                                   isOutput=False)
    eM = nc.declare_dram_parameter("eM", [128, 2, 128], F16, isOutput=False)
    sM = nc.declare_dram_parameter("sM", [128, 2, 8, 128], F16, isOutput=False)
    bb = nc.declare_dram_parameter("bb", [128, 2 * FC], F32, isOutput=False)
    c2 = nc.declare_dram_parameter("c2", [128, 2], F32, isOutput=False)
    oT = nc.declare_dram_parameter("oT", [NRN_C, TOK_C], F32, isOutput=True)

    with tile.TileContext(nc) as tc:
        with tc.tile_pool(name="wp", bufs=1) as wp, \
             tc.tile_pool(name="cst", bufs=1) as cst, \
             tc.tile_pool(name="xp", bufs=4) as xp, \
             tc.tile_pool(name="zb", bufs=6) as zb, \
             tc.tile_pool(name="tb", bufs=18) as tb, \
             tc.tile_pool(name="ob", bufs=2) as ob, \
             tc.tile_pool(name="zp", bufs=2, space="PSUM") as zp, \
             tc.tile_pool(name="hp", bufs=4, space="PSUM") as hp, \
             tc.tile_pool(name="op", bufs=2, space="PSUM") as op:

            # --- PE warm-up + ACT table preload during the input DMA window
            wu_f = cst.tile([128, 512], F32, tag="wuf")
            nc.vector.memset(wu_f[:], 0.001)
            wu = cst.tile([128, 512], F16, tag="wu")
            nc.vector.tensor_copy(wu[:], wu_f[:])
            nc.scalar.activation(wu_f[:, 0:8], wu_f[:, 8:16],
                                 mybir.ActivationFunctionType.Tanh)
            for _ in range(14):
                wu_ps = zp.tile([128, 512], F32, tag="z")
                nc.tensor.matmul(wu_ps[:], wu[:, 0:128], wu[:],
                                 start=True, stop=True)

            # --- inputs on the sync ring.  The first z group needs x0 +
            # w[fc0]; issue those first, split so the pieces spread across
            # the HW queues, then stream the rest of w.  Small consts go
            # on the scalar ring, which drains before tanh work starts.
            x0 = xp.tile([128, KC * 512], F16, tag="x")
            x_tiles = [x0]
            for nb in range(1, NB):
                x_tiles.append(xp.tile([128, KC * 512], F16, tag="x",
                                       name=f"x_{nb}"))
            w_all = wp.tile([128, FC, KC * 128], F16, tag="w")
            s_sb = cst.tile([128, 2, 8, 128], F16, tag="s")
            for i in range(4):     # w[fc0] in 4 pieces
                sl = bass.ds(i * 256, 256)
                nc.sync.dma_start(w_all[:, 0, sl], wT[:, 0, sl])
            for i in range(8):     # x0 in 8 pieces
                sl = bass.ds(i * 512, 512)
                nc.sync.dma_start(x0[:, sl], xT[:, 0, sl])
            for i in range(4):     # w[fc1]
                sl = bass.ds(i * 256, 256)
                nc.sync.dma_start(w_all[:, 1, sl], wT[:, 1, sl])
            for fc in (2, 3):      # earliest-needed chunks in halves
                for i in range(2):
                    sl = bass.ds(i * 512, 512)
                    nc.sync.dma_start(w_all[:, fc, sl], wT[:, fc, sl])
            for fc in range(4, FC):
                nc.sync.dma_start(w_all[:, fc], wT[:, fc])
                if fc == 5:   # s halves slot in behind the w chunks that the
                    nc.sync.dma_start(s_sb[:, 0], sM[:, 0])
                if fc == 9:   # first (second) cell type's S-matmuls need
                    nc.sync.dma_start(s_sb[:, 1], sM[:, 1])
            for nb in (1, 2, 3):   # x prefetches after the full w stream
                for i in range(4):
                    sl = bass.ds(i * 1024, 1024)
                    nc.sync.dma_start(x_tiles[nb][:, sl], xT[:, nb, sl])
            e_sb = cst.tile([128, 2, 128], F16, tag="e")
            nc.scalar.dma_start(e_sb[:], eM[:])
            bb_sb = cst.tile([128, 2 * FC], F32, tag="bb")
            nc.scalar.dma_start(bb_sb[:], bb[:])
            c2_sb = cst.tile([128, 2], F32, tag="c2")
            nc.scalar.dma_start(c2_sb[:], c2[:])

            # Software-pipelined epilogue: for each 128-feature chunk the
            # expansion (E) matmuls run one chunk behind the mains; the
            # reduction for a full (nb, grp) unit runs once all 8 of its
            # tanh tiles exist, so the PE never waits on the DVE cast or
            # the ScalarE tanh.
            unit_ths = {}                      # (nb, grp) -> [th] * 8
            e_stage = []                       # awaiting expansion
            s_stage = []                       # (unit, qslot) awaiting S
            o_next = [None]                    # pre-cleared o_ps for next unit
            qnow = [0]                         # global q-slot counter

            def alloc_clear_ops():
                # Pre-clear the next unit's PSUM bank on the DVE (one unit
                # ahead, so the PE never waits): with data == 0, the
                # reduction matmuls can all run start=False -- accumulate
                # and overwrite are then equivalent per element.
                t = op.tile([128, 512], F32, tag="o")
                nc.vector.memset(t[:], 0.0)
                return t

            def emit_expansion(item):
                nb, grp, q, tl, z_sb = item
                fc = grp * 4 + q
                ths = unit_ths.setdefault((nb, grp), [])
                for half in range(2):
                    ci = fc * 2 + half
                    # K=64 row-tiled pair: base_partition 0/64 auto-derives
                    # tile_position -> both halves run concurrently.
                    h_ps = hp.tile([128, 512], F32, tag="h")
                    nc.tensor.matmul(
                        h_ps[:],
                        e_sb[bass.ds(half * 64, 64), tl, :],
                        z_sb[bass.ds(half * 64, 64), :],
                        start=True, stop=True)
                    th = tb.tile([128, 512], F16, tag="t")
                    nc.scalar.activation(
                        th[:], h_ps[:],
                        mybir.ActivationFunctionType.Tanh,
                        bias=bb_sb[:, bass.ds(ci, 1)])
                    ths.append(th)
                if q == 3:
                    s_stage.append(
                        ((nb, grp, tl, unit_ths.pop((nb, grp))), qnow[0]))

            def emit_reduction(item):
                # Full reduction for one (nb, grp) unit: 8 th tiles, each
                # holding 16 neurons x 8 h on its partitions.  All 8 rr run
                # as M=32 column-tiled matmuls into the pre-zeroed bank:
                # each only touches its own 32-column group of the PE array,
                # so the 4 column groups execute concurrently (2 waves
                # instead of 8 serial slots).
                nb, grp, tl, ths = item
                if o_next[0] is None:
                    o_next[0] = alloc_clear_ops()
                o_ps = o_next[0]
                for rr in (0, 2, 4, 6, 1, 3, 5, 7):  # 2 waves of 4 col grps
                    j = rr // 2
                    nc.tensor.matmul(
                        o_ps[bass.ds(32 * j, 32), :],
                        s_sb[:, tl, rr, bass.ds(32 * j, 32)],
                        ths[rr][:],
                        start=False, stop=(rr == 7), skip_group_check=True,
                        tile_position=(0, 32 * j))
                o_next[0] = alloc_clear_ops()
                o_sb = ob.tile([128, 512], F32, tag="o")
                last = (nb == NB - 1 and grp == 3)
                if last:
                    # tail chain: use the idle ScalarE for the cb2 add and
                    # the lower-latency HWDGE sync ring, split over 4 queues
                    nc.scalar.activation(
                        o_sb[:], o_ps[:],
                        mybir.ActivationFunctionType.Identity,
                        bias=c2_sb[:, bass.ds(tl, 1)])
                    for i in range(4):
                        nc.sync.dma_start(
                            oT[bass.ds(grp * 128, 128),
                               bass.ds(nb * 512 + i * 128, 128)],
                            o_sb[:, bass.ds(i * 128, 128)])
                else:
                    nc.vector.tensor_scalar_add(
                        o_sb[:], o_ps[:], c2_sb[:, bass.ds(tl, 1)])
                    nc.gpsimd.dma_start(
                        oT[bass.ds(grp * 128, 128), bass.ds(nb * 512, 512)],
                        o_sb[:])

            for nb in range(NB):
                x_nb = x_tiles[nb]

                for grp in range(4):          # 4 fc chunks -> 128 neurons
                    tl = (grp * 4) // TL
                    for q in range(4):
                        fc = grp * 4 + q
                        z_ps = zp.tile([128, 512], F32, tag="z")
                        for k in range(KC):
                            nc.tensor.matmul(
                                z_ps[:],
                                w_all[:, fc, bass.ds(k * 128, 128)],
                                x_nb[:, bass.ds(k * 512, 512)],
                                start=(k == 0), stop=(k == KC - 1))
                        z_sb = zb.tile([128, 512], F16, tag="z")
                        nc.vector.tensor_copy(z_sb[:], z_ps[:])

                        # reduction lags its unit's last tanh by >=2 q-slots
                        if s_stage and qnow[0] >= s_stage[0][1] + 2:
                            emit_reduction(s_stage.pop(0)[0])
                        # expansions go out 2 fc at a time (one mode switch
                        # amortized over 4 row-tiled K=64 matmul pairs)
                        if len(e_stage) >= 4:
                            emit_expansion(e_stage.pop(0))
                            emit_expansion(e_stage.pop(0))
                        e_stage.append((nb, grp, q, tl, z_sb))
                        qnow[0] += 1
                        # flush eagerly near the end so the final tanh
                        # batch overlaps the remaining reductions
                        if nb == NB - 1 and grp == 3 and q in (1, 3):
                            while e_stage:
                                emit_expansion(e_stage.pop(0))
                                emit_expansion(e_stage.pop(0))

            while e_stage:
                emit_expansion(e_stage.pop(0))
                emit_expansion(e_stage.pop(0))
                if s_stage:
                    emit_reduction(s_stage.pop(0)[0])
            while s_stage:
                emit_reduction(s_stage.pop(0)[0])

    nc.compile()
    return nc


def _host_prep(x, weight, bias, cw1, cb1, cw2, cb2):
    """Build the 8 per-core input maps (all host-side numpy)."""
    x2 = np.ascontiguousarray(x, dtype=np.float32).reshape(NTOK, DIN)
    weight = np.asarray(weight, dtype=np.float32)
    bias = np.asarray(bias, dtype=np.float32)
    cw1 = np.asarray(cw1, dtype=np.float32)   # (T, A, H)
    cb1 = np.asarray(cb1, dtype=np.float32)   # (T, H)
    cw2 = np.asarray(cw2, dtype=np.float32)   # (T, H)
    cb2 = np.asarray(cb2, dtype=np.float32)   # (T,)

    # xT[p, nb, k*512 + j] = x2[tok0 + nb*512 + j, k*128 + p]
    xT_all = []
    for i in range(DP):
        blk = x2[i * TOK_C:(i + 1) * TOK_C]            # (TOK_C, DIN)
        t = blk.T.reshape(KC, 128, NB, 512)            # (k, p, nb, j)
        t = t.transpose(1, 2, 0, 3).reshape(128, NB, KC * 512)
        xT_all.append(np.ascontiguousarray(t, dtype=np.float16))

    # wT[p, fc, k*128 + f] = W[j*FEAT_C + fc*128 + f, k*128 + p]
    wT_all = []
    for j in range(TP):
        wj = weight[j * FEAT_C:(j + 1) * FEAT_C]       # (FEAT_C, DIN)
        t = wj.T.reshape(KC, 128, FC, 128)             # (k, p, fc, f)
        t = t.transpose(1, 2, 0, 3).reshape(128, FC, KC * 128)
        wT_all.append(np.ascontiguousarray(t, dtype=np.float16))

    # E[t]: (g*4+a, g16*8+h) block-diag cw1; S[t]: (g*8+h, g') block-diag cw2
    e_all, s_all, bb_all, c2_all = [], [], [], []
    for j in range(TP):
        eMj = np.zeros((128, 2, 128), np.float32)
        sMj = np.zeros((128, 2, 8, 128), np.float32)
        for tl in range(2):
            t = 2 * j + tl
            for g16 in range(16):   # K=64 expansion block, doubled on rows
                for a in range(A):
                    for h in range(H):
                        v = cw1[t, a, h]
                        eMj[g16 * 4 + a, tl, g16 * 8 + h] = v
                        eMj[64 + g16 * 4 + a, tl, g16 * 8 + h] = v
            for rr in range(8):
                for g in range(16):
                    for h in range(H):
                        sMj[g * 8 + h, tl, rr, rr * 16 + g] = cw2[t, h]
        e_all.append(eMj.astype(np.float16))
        s_all.append(sMj.astype(np.float16))

        # bb[f2=(n_loc, h)] = sum_a cw1[t,a,h]*bias[t*1024+g_t*4+a] + cb1[t,h]
        nl = np.arange(NRN_C)
        t_of = (j * NRN_C + nl) // G                   # cell type per neuron
        gt = (j * NRN_C + nl) % G                      # group within type
        bias_ga = bias.reshape(T, G, A)[t_of, gt]      # (NRN_C, A)
        bbv = np.einsum('na,nah->nh', bias_ga, cw1[t_of]) + cb1[t_of]
        bb_all.append(np.ascontiguousarray(
            bbv.reshape(NRN_C * H).reshape(2 * FC, 128).T))

        c2m = np.zeros((128, 2), np.float32)
        for tl in range(2):
            c2m[:, tl] = cb2[2 * j + tl]
        c2_all.append(c2m)

    in_maps = []
    for c in range(8):
        i, j = c // TP, c % TP
        in_maps.append({
            "xT": xT_all[i], "wT": wT_all[j], "eM": e_all[j],
            "sM": s_all[j], "bb": bb_all[j], "c2": c2_all[j],
        })
    return in_maps


def kernel(x, weight, bias, cw1, cb1, cw2, cb2):
    in_maps = _host_prep(x, weight, bias, cw1, cb1, cw2, cb2)
    if not _NC_CACHE:
        _NC_CACHE.append(_build_nc())
    nc = _NC_CACHE[0]
    try:
        res = run_bass_kernel_spmd(nc, in_maps, list(range(8)))
    except Exception:
        # transient NRT device faults have been observed once after crashed
        # runs; a clean retry in the same process recovers
        res = run_bass_kernel_spmd(nc, in_maps, list(range(8)))
    out = np.empty((NTOK, DOUT), np.float32)
    for c in range(8):
        i, j = c // TP, c % TP
        oc = res.results[c]["oT"]                      # (NRN_C, TOK_C)
        out[i * TOK_C:(i + 1) * TOK_C, j * NRN_C:(j + 1) * NRN_C] = oc.T
    return out.reshape(B, S, DOUT)


# revision 35
# speedup vs baseline: 1.0350x; 1.0162x over previous
"""Trainium2 Bass kernel for DenseLayerWithComplexNeurons.

Reference computation (B=8, S=1024, DIN=1024, DOUT=1024, A=4, T=4, H=8):
    z = x @ W.T + bias                      # (B,S, A*DOUT)
    z -> (B,S,T,G,A), G = DOUT//T = 256
    h = tanh(z @ cw1[t] + cb1[t])           # (B,S,T,G,H)
    o = h @ cw2[t] + cb2[t]                 # (B,S,T,G) -> (B,S,DOUT)

Sharding: 8 cores = 4 token blocks (2048 tokens each) x 2 feature halves
(2048 W-rows / 512 neurons each).  All compute runs in a transposed layout
(features on partitions, tokens on the free dim) so the tiny per-neuron
MLPs become small constant matmuls on the tensor engine:
  - expansion E[t]: (g,a) -> (g,h) block-diagonal with cw1
  - reduction S[t]: (g,h) -> (g)   block-diagonal with cw2
The linear bias and cb1 are folded through cw1 into a single per-feature
bias bb added by the scalar engine inside tanh.  All matmul operands are
fp16: same PE column rate as fp32r, but LDWEIGHTS uses fast-weight-load
(2 elems per 32b read) so the weight loads hide behind the matmuls, and
DMA traffic halves.
"""

import numpy as np

import concourse.bass as bass  # noqa: F401  (bass types via bacc)
import concourse.mybir as mybir
import concourse.tile as tile
from concourse import bacc
from concourse.bass_utils import run_bass_kernel_spmd

F32 = mybir.dt.float32
F16 = mybir.dt.float16

B, S, DIN, DOUT, A, T, H = 8, 1024, 1024, 1024, 4, 4, 8
G = DOUT // T                     # 256 neurons per cell type
NTOK = B * S                      # 8192 tokens
DP, TP = 4, 2                     # token blocks x feature halves
TOK_C = NTOK // DP                # 2048 tokens per core
NRN_C = DOUT // TP                # 512 neurons per core
FEAT_C = A * NRN_C                # 2048 A-expanded features per core
KC = DIN // 128                   # 8 contraction chunks
NB = TOK_C // 512                 # 4 token sub-blocks per core
FC = FEAT_C // 128                # 16 feature chunks per core
TL = FC // 2                      # 8 feature chunks per cell type


_NC_CACHE = []


def _build_nc():
    nc = bacc.Bacc("TRN2", target_bir_lowering=False, debug=False, num_devices=8)

    # layouts chosen so every DMA slice is contiguous per partition:
    # xT[p, nb, k*512 + j] and wT[p, fc, k*128 + f]
    xT = nc.declare_dram_parameter("xT", [128, NB, KC * 512], F16,
                                   isOutput=False)
    wT = nc.declare_dram_parameter("wT", [128, FC, KC * 128], F16,
                                   isOutput=False)
    eM = nc.declare_dram_parameter("eM", [128, 2, 128], F16, isOutput=False)
    sM = nc.declare_dram_parameter("sM", [128, 2, 8, 128], F16, isOutput=False)
    bb = nc.declare_dram_parameter("bb", [128, 2 * FC], F32, isOutput=False)
    c2 = nc.declare_dram_parameter("c2", [128, 2], F32, isOutput=False)
    oT = nc.declare_dram_parameter("oT", [NRN_C, TOK_C], F32, isOutput=True)

    with tile.TileContext(nc) as tc:
        with tc.tile_pool(name="wp", bufs=1) as wp, \
             tc.tile_pool(name="cst", bufs=1) as cst, \
             tc.tile_pool(name="xp", bufs=4) as xp, \
             tc.tile_pool(name="zb", bufs=6) as zb, \
             tc.tile_pool(name="tb", bufs=18) as tb, \
             tc.tile_pool(name="ob", bufs=2) as ob, \
             tc.tile_pool(name="zp", bufs=2, space="PSUM") as zp, \
             tc.tile_pool(name="hp", bufs=4, space="PSUM") as hp, \
             tc.tile_pool(name="op", bufs=2, space="PSUM") as op:

            # --- PE warm-up + ACT table preload during the input DMA window
            wu_f = cst.tile([128, 512], F32, tag="wuf")
            nc.vector.memset(wu_f[:], 0.001)
            wu = cst.tile([128, 512], F16, tag="wu")
            nc.vector.tensor_copy(wu[:], wu_f[:])
            nc.scalar.activation(wu_f[:, 0:8], wu_f[:, 8:16],
                                 mybir.ActivationFunctionType.Tanh)
            for _ in range(14):
                wu_ps = zp.tile([128, 512], F32, tag="z")
                nc.tensor.matmul(wu_ps[:], wu[:, 0:128], wu[:],
                                 start=True, stop=True)

            # --- inputs on the sync ring.  The first z group needs x0 +
            # w[fc0] + w[fc1]; issue those first, split so the pieces
            # spread across the HW queues, then stream the rest of w, then
            # the x1..x3 prefetches (needed much later).  Small consts go
            # on the scalar ring, which drains before tanh work starts.
            x0 = xp.tile([128, KC * 512], F16, tag="x")
            x_tiles = [x0]
            for nb in range(1, NB):
                x_tiles.append(xp.tile([128, KC * 512], F16, tag="x",
                                       name=f"x_{nb}"))
            w_all = wp.tile([128, FC, KC * 128], F16, tag="w")
            s_sb = cst.tile([128, 2, 8, 128], F16, tag="s")
            for i in range(4):     # w[fc0] in 4 pieces
                sl = bass.ds(i * 256, 256)
                nc.sync.dma_start(w_all[:, 0, sl], wT[:, 0, sl])
            for i in range(8):     # x0 in 8 pieces
                sl = bass.ds(i * 512, 512)
                nc.sync.dma_start(x0[:, sl], xT[:, 0, sl])
            for i in range(4):     # w[fc1]
                sl = bass.ds(i * 256, 256)
                nc.sync.dma_start(w_all[:, 1, sl], wT[:, 1, sl])
            for fc in (2, 3):      # earliest-needed chunks in halves
                for i in range(2):
                    sl = bass.ds(i * 512, 512)
                    nc.sync.dma_start(w_all[:, fc, sl], wT[:, fc, sl])
            for fc in range(4, FC):
                nc.sync.dma_start(w_all[:, fc], wT[:, fc])
                if fc == 5:   # s halves slot in behind the w chunks that the
                    nc.sync.dma_start(s_sb[:, 0], sM[:, 0])
                if fc == 9:   # first (second) cell type's S-matmuls need
                    nc.sync.dma_start(s_sb[:, 1], sM[:, 1])
            for nb in (1, 2, 3):   # x prefetches after the full w stream
                for i in range(4):
                    sl = bass.ds(i * 1024, 1024)
                    nc.sync.dma_start(x_tiles[nb][:, sl], xT[:, nb, sl])
            e_sb = cst.tile([128, 2, 128], F16, tag="e")
            nc.scalar.dma_start(e_sb[:], eM[:])
            bb_sb = cst.tile([128, 2 * FC], F32, tag="bb")
            nc.scalar.dma_start(bb_sb[:], bb[:])
            c2_sb = cst.tile([128, 2], F32, tag="c2")
            nc.scalar.dma_start(c2_sb[:], c2[:])

            # Software-pipelined epilogue: for each 128-feature chunk the
            # expansion (E) matmuls run one chunk behind the mains; the
            # reduction for a full (nb, grp) unit runs once all 8 of its
            # tanh tiles exist, so the PE never waits on the DVE cast or
            # the ScalarE tanh.
            unit_ths = {}                      # (nb, grp) -> [th] * 8
            e_stage = []                       # awaiting expansion
            s_stage = []                       # (unit, qslot) awaiting S
            o_next = [None]                    # pre-cleared o_ps for next unit
            qnow = [0]                         # global q-slot counter

            def alloc_clear_ops():
                # Pre-clear the next unit's PSUM bank on the DVE (one unit
                # ahead, so the PE never waits): with data == 0, the
                # reduction matmuls can all run start=False -- accumulate
                # and overwrite are then equivalent per element.
                t = op.tile([128, 512], F32, tag="o")
                nc.vector.memset(t[:], 0.0)
                return t

            def emit_expansion(item):
                nb, grp, q, tl, z_sb = item
                fc = grp * 4 + q
                ths = unit_ths.setdefault((nb, grp), [])
                for half in range(2):
                    ci = fc * 2 + half
                    # K=64 row-tiled pair: base_partition 0/64 auto-derives
                    # tile_position -> both halves run concurrently.
                    h_ps = hp.tile([128, 512], F32, tag="h")
                    nc.tensor.matmul(
                        h_ps[:],
                        e_sb[bass.ds(half * 64, 64), tl, :],
                        z_sb[bass.ds(half * 64, 64), :],
                        start=True, stop=True)
                    th = tb.tile([128, 512], F16, tag="t")
                    nc.scalar.activation(
                        th[:], h_ps[:],
                        mybir.ActivationFunctionType.Tanh,
                        bias=bb_sb[:, bass.ds(ci, 1)])
                    ths.append(th)
                if q == 3:
                    s_stage.append(
                        ((nb, grp, tl, unit_ths.pop((nb, grp))), qnow[0]))

            def emit_reduction(item):
                # Full reduction for one (nb, grp) unit: 8 th tiles, each
                # holding 16 neurons x 8 h on its partitions.  All 8 rr run
                # as M=32 column-tiled matmuls into the pre-zeroed bank:
                # each only touches its own 32-column group of the PE array,
                # so the 4 column groups execute concurrently (2 waves
                # instead of 8 serial slots).
                nb, grp, tl, ths = item
                if o_next[0] is None:
                    o_next[0] = alloc_clear_ops()
                o_ps = o_next[0]
                for rr in (0, 2, 4, 6, 1, 3, 5, 7):  # 2 waves of 4 col grps
                    j = rr // 2
                    nc.tensor.matmul(
                        o_ps[bass.ds(32 * j, 32), :],
                        s_sb[:, tl, rr, bass.ds(32 * j, 32)],
                        ths[rr][:],
                        start=False, stop=(rr == 7), skip_group_check=True,
                        tile_position=(0, 32 * j))
                o_next[0] = alloc_clear_ops()
                o_sb = ob.tile([128, 512], F32, tag="o")
                last = (nb == NB - 1 and grp == 3)
                if last:
                    # tail chain: use the idle ScalarE for the cb2 add and
                    # the lower-latency HWDGE sync ring, split over 4 queues
                    nc.scalar.activation(
                        o_sb[:], o_ps[:],
                        mybir.ActivationFunctionType.Identity,
                        bias=c2_sb[:, bass.ds(tl, 1)])
                    for i in range(4):
                        nc.sync.dma_start(
                            oT[bass.ds(grp * 128, 128),
                               bass.ds(nb * 512 + i * 128, 128)],
                            o_sb[:, bass.ds(i * 128, 128)])
                else:
                    nc.vector.tensor_scalar_add(
                        o_sb[:], o_ps[:], c2_sb[:, bass.ds(tl, 1)])
                    nc.gpsimd.dma_start(
                        oT[bass.ds(grp * 128, 128), bass.ds(nb * 512, 512)],
                        o_sb[:])

            for nb in range(NB):
                x_nb = x_tiles[nb]

                for grp in range(4):          # 4 fc chunks -> 128 neurons
                    tl = (grp * 4) // TL
                    for q in range(4):
                        fc = grp * 4 + q
                        z_ps = zp.tile([128, 512], F32, tag="z")
                        for k in range(KC):
                            nc.tensor.matmul(
                                z_ps[:],
                                w_all[:, fc, bass.ds(k * 128, 128)],
                                x_nb[:, bass.ds(k * 512, 512)],
                                start=(k == 0), stop=(k == KC - 1))
                        z_sb = zb.tile([128, 512], F16, tag="z")
                        nc.vector.tensor_copy(z_sb[:], z_ps[:])

                        # reduction lags its unit's last tanh by >=2 q-slots
                        if s_stage and qnow[0] >= s_stage[0][1] + 2:
                            emit_reduction(s_stage.pop(0)[0])
                        # expansions go out 2 fc at a time (one mode switch
                        # amortized over 4 row-tiled K=64 matmul pairs)
                        if len(e_stage) >= 4:
                            emit_expansion(e_stage.pop(0))
                            emit_expansion(e_stage.pop(0))
                        e_stage.append((nb, grp, q, tl, z_sb))
                        qnow[0] += 1
                        # flush eagerly near the end so the final tanh
                        # batch overlaps the remaining reductions
                        if nb == NB - 1 and grp == 3 and q in (1, 3):
                            while e_stage:
                                emit_expansion(e_stage.pop(0))
                                emit_expansion(e_stage.pop(0))

            while e_stage:
                emit_expansion(e_stage.pop(0))
                emit_expansion(e_stage.pop(0))
                if s_stage:
                    emit_reduction(s_stage.pop(0)[0])
            while s_stage:
                emit_reduction(s_stage.pop(0)[0])

    nc.compile()
    return nc


def _host_prep(x, weight, bias, cw1, cb1, cw2, cb2):
    """Build the 8 per-core input maps (all host-side numpy)."""
    x2 = np.ascontiguousarray(x, dtype=np.float32).reshape(NTOK, DIN)
    weight = np.asarray(weight, dtype=np.float32)
    bias = np.asarray(bias, dtype=np.float32)
    cw1 = np.asarray(cw1, dtype=np.float32)   # (T, A, H)
    cb1 = np.asarray(cb1, dtype=np.float32)   # (T, H)
    cw2 = np.asarray(cw2, dtype=np.float32)   # (T, H)
    cb2 = np.asarray(cb2, dtype=np.float32)   # (T,)

    # xT[p, nb, k*512 + j] = x2[tok0 + nb*512 + j, k*128 + p]
    xT_all = []
    for i in range(DP):
        blk = x2[i * TOK_C:(i + 1) * TOK_C]            # (TOK_C, DIN)
        t = blk.T.reshape(KC, 128, NB, 512)            # (k, p, nb, j)
        t = t.transpose(1, 2, 0, 3).reshape(128, NB, KC * 512)
        xT_all.append(np.ascontiguousarray(t, dtype=np.float16))

    # wT[p, fc, k*128 + f] = W[j*FEAT_C + fc*128 + f, k*128 + p]
    wT_all = []
    for j in range(TP):
        wj = weight[j * FEAT_C:(j + 1) * FEAT_C]       # (FEAT_C, DIN)
        t = wj.T.reshape(KC, 128, FC, 128)             # (k, p, fc, f)
        t = t.transpose(1, 2, 0, 3).reshape(128, FC, KC * 128)
        wT_all.append(np.ascontiguousarray(t, dtype=np.float16))

    # E[t]: (g*4+a, g16*8+h) block-diag cw1; S[t]: (g*8+h, g') block-diag cw2
    e_all, s_all, bb_all, c2_all = [], [], [], []
    for j in range(TP):
        eMj = np.zeros((128, 2, 128), np.float32)
        sMj = np.zeros((128, 2, 8, 128), np.float32)
        for tl in range(2):
            t = 2 * j + tl
            for g16 in range(16):   # K=64 expansion block, doubled on rows
                for a in range(A):
                    for h in range(H):
                        v = cw1[t, a, h]
                        eMj[g16 * 4 + a, tl, g16 * 8 + h] = v
                        eMj[64 + g16 * 4 + a, tl, g16 * 8 + h] = v
            for rr in range(8):
                for g in range(16):
                    for h in range(H):
                        sMj[g * 8 + h, tl, rr, rr * 16 + g] = cw2[t, h]
        e_all.append(eMj.astype(np.float16))
        s_all.append(sMj.astype(np.float16))

        # bb[f2=(n_loc, h)] = sum_a cw1[t,a,h]*bias[t*1024+g_t*4+a] + cb1[t,h]
        nl = np.arange(NRN_C)
        t_of = (j * NRN_C + nl) // G                   # cell type per neuron
        gt = (j * NRN_C + nl) % G                      # group within type
        bias_ga = bias.reshape(T, G, A)[t_of, gt]      # (NRN_C, A)
        bbv = np.einsum('na,nah->nh', bias_ga, cw1[t_of]) + cb1[t_of]
        bb_all.append(np.ascontiguousarray(
            bbv.reshape(NRN_C * H).reshape(2 * FC, 128).T))

        c2m = np.zeros((128, 2), np.float32)
        for tl in range(2):
            c2m[:, tl] = cb2[2 * j + tl]
        c2_all.append(c2m)

    in_maps = []
    for c in range(8):
        i, j = c // TP, c % TP
        in_maps.append({
            "xT": xT_all[i], "wT": wT_all[j], "eM": e_all[j],
            "sM": s_all[j], "bb": bb_all[j], "c2": c2_all[j],
        })
    return in_maps


def kernel(x, weight, bias, cw1, cb1, cw2, cb2):
    in_maps = _host_prep(x, weight, bias, cw1, cb1, cw2, cb2)
    if not _NC_CACHE:
        _NC_CACHE.append(_build_nc())
    nc = _NC_CACHE[0]
    try:
        res = run_bass_kernel_spmd(nc, in_maps, list(range(8)))
    except Exception:
        # transient NRT device faults have been observed once after crashed
        # runs; a clean retry in the same process recovers
        res = run_bass_kernel_spmd(nc, in_maps, list(range(8)))
    out = np.empty((NTOK, DOUT), np.float32)
    for c in range(8):
        i, j = c // TP, c % TP
        oc = res.results[c]["oT"]                      # (NRN_C, TOK_C)
        out[i * TOK_C:(i + 1) * TOK_C, j * NRN_C:(j + 1) * NRN_C] = oc.T
    return out.reshape(B, S, DOUT)
